# revision 65
# baseline (speedup 1.0000x reference)
"""Trainium2 Bass kernel for MultiHeadSelfAttention with RoPE.

Problem: x[2, 2048, 1024] @ W_qkv[1024, 3072] -> rope(q,k) -> softmax(q k^T/8) v
         -> out @ W_out[1024, 1024].

Sharding (8 cores): batch (2-way) x head-group (4-way, 4 heads each).
Each core computes a partial output [2048, 1024] = attnout_heads @ W_out_rows;
host sums the 4 head-group partials per batch.

All matmul operands are bf16 (PSUM accumulation fp32), which halves DMA/SBUF
traffic vs fp32 and enables fast weight load on the PE.  Expected rel err
~3e-3 (bf16 input rounding), well under the 2e-2 gate.

On-core dataflow is fully "transposed" so the PE never needs a transpose:
  qT,kT[c, s] = sum_e W[e, c] * xT[e, s]   (lhsT = W slice, rhs = xT)
  rot = Mswap @ qT (PE), q' = qT*cos + rot*sin_signed (DVE)
  scoresT[sk, sq] = sum_d kT[d, sk] qT[d, sq]  (per head, K=128 zero-padded)
  attnT = exp(scoresT/8) on ScalarE over TWO sk tiles at once ([128, 2048])
  oT[d, sq] += sum_sk v[sk, d] attnT[sk, sq]   (M=65: 64 dims + denom row)
  attnout = oT * (1/denom)  -> out_partial[s, e] = attnoutT.T @ W_out_rows

Engine schedule: the attention stretch is paced by ScalarE (exp); PE matmuls
for the second head-pair's projection and the first output-projection half
are interleaved ("fillers") into the exp-paced gaps so the PE never idles.
"""

import sys
from contextlib import ExitStack

if "/opt/trn_rl_repo" not in sys.path:
    sys.path.insert(0, "/opt/trn_rl_repo")

import numpy as np

B, S, E = 2, 2048, 1024
ATT = 1024
H = 16
D = 64
HG = 4            # head groups (cores per batch)
HPG = H // HG     # heads per core = 4
PAIRS = HPG // 2  # head pairs per core = 2
ROPE_THETA = 10000.0
N_CORES = 8

SQ = 1024         # sq chunk width (ch = 0, 1)
N_SKP = 8         # sk-pairs per (head, chunk); each pair = 2 sk tiles of 128

_BUILT = {}


def _build_program():
    import concourse.bacc as bacc
    import concourse.tile as tile
    import concourse.mybir as mybir

    f32 = mybir.dt.float32
    bf16 = mybir.dt.bfloat16
    AF = mybir.ActivationFunctionType

    nc = bacc.Bacc(
        "TRN2",
        target_bir_lowering=False,
        debug=False,
        enable_asserts=False,
        num_devices=N_CORES,
    )

    xT = nc.dram_tensor("xT", [E, S], bf16, kind="ExternalInput").ap()
    w_qk = nc.dram_tensor("w_qk", [E, 2 * HPG * D], bf16, kind="ExternalInput").ap()
    w_v = nc.dram_tensor("w_v", [128, E // 128, HPG * D], bf16, kind="ExternalInput").ap()
    w_o = nc.dram_tensor("w_o", [HPG * D, E], bf16, kind="ExternalInput").ap()
    cos_t = nc.dram_tensor("cos_t", [128, S], bf16, kind="ExternalInput").ap()
    sin_t = nc.dram_tensor("sin_t", [128, S], bf16, kind="ExternalInput").ap()
    mswap = nc.dram_tensor("mswap", [128, 128], bf16, kind="ExternalInput").ap()
    ones_in = nc.dram_tensor("ones_in", [1, 64], bf16, kind="ExternalInput").ap()
    out = nc.dram_tensor("out", [S, E], bf16, kind="ExternalOutput").ap()

    EK = E // 128  # 8 contraction tiles over embedding dim

    with tile.TileContext(nc) as tc:
        with (
            tc.tile_pool(name="const", bufs=1) as constp,
            tc.tile_pool(name="qkT", bufs=1) as qkTp,
            tc.tile_pool(name="vsb", bufs=1) as vp,
            tc.tile_pool(name="attnout", bufs=1) as aop,
            tc.tile_pool(name="wsb", bufs=1) as wp,
            tc.tile_pool(name="xsb", bufs=1) as xp,
            tc.tile_pool(name="ropes", bufs=3) as ropep,
            tc.tile_pool(name="exps", bufs=4) as expp,
            tc.tile_pool(name="normp", bufs=2) as nmp,
            tc.tile_pool(name="outsb", bufs=4) as osbp,
        ):
            msw_sb = constp.tile([128, 128], bf16, tag="msw")
            onesrow = constp.tile([65, 64], bf16, tag="onesrow")
            cos_sb = constp.tile([128, S], bf16, tag="cos")
            sin_sb = constp.tile([128, S], bf16, tag="sin")

            # k' per pair: [128, S] (rows 0:64 head A dims, 64:128 head B).
            # q' split into two zero-padded [128, S] tensors so score matmuls
            # contract over K=128: qzlo = [q'_A | 0], qzhi = [0 | q'_B].
            qzlo = [qkTp.tile([128, S], bf16, tag=f"qzlo{g}", name=f"qzlo{g}") for g in range(PAIRS)]
            qzhi = [qkTp.tile([128, S], bf16, tag=f"qzhi{g}", name=f"qzhi{g}") for g in range(PAIRS)]
            kT = [qkTp.tile([128, S], bf16, tag=f"kT{g}", name=f"kT{g}") for g in range(PAIRS)]
            # v natural + aug ones column, 4 heads: head h occupies cols
            # [65h, 65h+64) = v, col 65h+64 = ones (softmax denominator row)
            v_c = vp.tile([128, S // 128, 4, 65], bf16, tag="vc", name="vc")
            # normalized attention output per pair [128 (pair dims), S]
            att_o = [aop.tile([128, S], bf16, tag=f"ao{g}", name=f"ao{g}") for g in range(PAIRS)]
            wo_sb = [wp.tile([128, E], bf16, tag=f"wo{g}", name=f"wo{g}") for g in range(PAIRS)]
            wqk_sb = [wp.tile([128, 2 * HPG * D], bf16, tag=f"wqk{e}", name=f"wqk{e}") for e in range(EK)]
            wv_sb = wp.tile([128, EK, HPG * D], bf16, tag="wv", name="wv")
            xt_sb = [xp.tile([128, S], bf16, tag=f"xt{e}", name=f"xt{e}") for e in range(EK)]

            # zero halves of the padded q tensors; ones columns of v
            for g in range(PAIRS):
                nc.vector.memset(qzlo[g][64:128, :], 0.0)
                nc.vector.memset(qzhi[g][0:64, :], 0.0)
            for h in range(4):
                nc.vector.memset(v_c[:, :, h, 64], 1.0)

            # ---- DMA order: first-needed first ----
            nc.sync.dma_start(wqk_sb[0][:], w_qk[0:128, :])
            nc.sync.dma_start(xt_sb[0][:], xT[0:128, :])
            nc.sync.dma_start(msw_sb[:], mswap[:])
            for e in range(1, EK):
                nc.sync.dma_start(wqk_sb[e][:], w_qk[128 * e : 128 * (e + 1), :])
                nc.sync.dma_start(xt_sb[e][:], xT[128 * e : 128 * (e + 1), :])
            # wv (one transfer, host pre-shuffled to [p, e, c]) right after
            # the xt stream; cos/sin follow (first rope tail tolerates the
            # extra ~3us, the boot's xt7 does not)
            nc.sync.dma_start(wv_sb[:], w_v[:])
            nc.sync.dma_start(cos_sb[:], cos_t[:])
            nc.sync.dma_start(sin_sb[:], sin_t[:])
            nc.sync.dma_start(onesrow[64:65, :], ones_in[:])
            for g in range(PAIRS):
                nc.sync.dma_start(wo_sb[g][:], w_o[128 * g : 128 * (g + 1), :])

            # ---------------- projection + rope ----------------
            rope_pend = []

            def rope_tail(evac_on_act):
                (g_, dest, sl, raw) = rope_pend.pop(0)
                rp = mps.tile([128, 512], f32, tag="m", name="rp")
                nc.tensor.matmul(rp[:], msw_sb[:], raw[:], start=True, stop=True)
                t2 = ropep.tile([128, 512], bf16, tag="t2")
                nc.gpsimd.tensor_tensor(t2[:], raw[:], cos_sb[:, sl], mybir.AluOpType.mult)
                t1 = ropep.tile([128, 512], bf16, tag="t1")
                nc.vector.tensor_mul(t1[:], rp[:], sin_sb[:, sl])
                if dest is None:
                    nc.vector.tensor_add(qzlo[g_][0:64, sl], t1[0:64, :], t2[0:64, :])
                    nc.gpsimd.tensor_tensor(
                        qzhi[g_][64:128, sl], t1[64:128, :], t2[64:128, :],
                        mybir.AluOpType.add,
                    )
                else:
                    nc.vector.tensor_add(dest[:, sl], t1[:], t2[:])

            def proj_chunk_mms(g, ti, c):
                """Closures for one (q|k, chunk): 8 accumulating MMs + evac.
                The PSUM tile is created inside the first closure so pool-slot
                rotation order matches emission order."""
                sl = slice(512 * c, 512 * (c + 1))
                coff = ti * HPG * D + 128 * g
                box = {}

                def mm(e):
                    def run():
                        if e == 0:
                            box["pp"] = mps.tile([128, 512], f32, tag="m", name="pp")
                        nc.tensor.matmul(
                            box["pp"][:],
                            wqk_sb[e][:, coff : coff + 128],
                            xt_sb[e][:, sl],
                            start=(e == 0),
                            stop=(e == EK - 1),
                        )

                    return run

                dest = None if ti == 0 else kT[g]

                def evac():
                    raw = ropep.tile([128, 512], bf16, tag="raw", bufs=9)
                    nc.vector.tensor_copy(raw[:], box["pp"][:])
                    rope_pend.append((g, dest, sl, raw))

                return [mm(e) for e in range(EK)] + [evac]

            def v_chunk_mms(st, evac_on_act):
                box = {}

                def mm(e):
                    def run():
                        if e == 0:
                            box["vps"] = mps.tile([128, 4, 64], f32, tag="m", name="vps")
                        nc.tensor.matmul(
                            box["vps"][:],
                            xt_sb[e][:, 128 * st : 128 * (st + 1)],
                            wv_sb[:, e, :],
                            start=(e == 0),
                            stop=(e == EK - 1),
                        )

                    return run

                def evac():
                    dst = v_c[:, st, :, 0:64]
                    src = box["vps"][:]
                    if evac_on_act:
                        nc.scalar.copy(dst, src)
                    else:
                        nc.vector.tensor_copy(dst, src)

                return [mm(e) for e in range(EK)] + [evac]

            # ---- boot: pair-0 q/k projection, e-major over 8 PSUM banks.
            # Each arriving (wqk, xt) tile pair feeds 8 matmuls (one per
            # output chunk), so the PE tracks the input DMA stream instead
            # of idling behind the first chunk's full K-accumulation. ----
            with tc.tile_pool(name="boot", bufs=1, space="PSUM") as bootp:
                combos = [(ti, c) for ti in (0, 1) for c in range(S // 512)]
                boots = [
                    bootp.tile([128, 512], f32, tag=f"b{i}", name=f"bt{i}")
                    for i in range(len(combos))
                ]
                for e in range(EK):
                    for i, (ti, c) in enumerate(combos):
                        coff = ti * HPG * D
                        nc.tensor.matmul(
                            boots[i][:],
                            wqk_sb[e][:, coff : coff + 128],
                            xt_sb[e][:, 512 * c : 512 * (c + 1)],
                            start=(e == 0),
                            stop=(e == EK - 1),
                        )
                for i, (ti, c) in enumerate(combos):
                    raw = ropep.tile([128, 512], bf16, tag="raw", bufs=9)
                    if i % 2 == 0:
                        nc.scalar.copy(raw[:], boots[i][:])
                    else:
                        nc.vector.tensor_copy(raw[:], boots[i][:])
                    rope_pend.append(
                        (0, None if ti == 0 else kT[0], slice(512 * c, 512 * (c + 1)), raw)
                    )

            # attention-phase PSUM: s 2x2 banks + oT 2 banks + misc 2 banks
            _psum_stack = ExitStack()
            sps = _psum_stack.enter_context(tc.tile_pool(name="sps", bufs=2, space="PSUM"))
            ops = _psum_stack.enter_context(tc.tile_pool(name="ops", bufs=1, space="PSUM"))
            mps = _psum_stack.enter_context(tc.tile_pool(name="mps", bufs=2, space="PSUM"))

            # v projection for all heads (needed from the first attnv);
            # pair-0 rope tails drain alongside, one per st
            for st in range(S // 128):
                for op in v_chunk_mms(st, True):
                    op()
                if rope_pend:
                    rope_tail(True)
            while rope_pend:
                rope_tail(True)

            # ---- filler queue: pair-1's projection, emitted inside
            # attention's exp-paced gaps (rope tails lag one chunk so the
            # `raw` slot rotation stays legal) ----
            fillers = []
            filler_chunks = [(1, ti, c) for ti in (0, 1) for c in range(S // 512)]
            for i, (g_, ti, c) in enumerate(filler_chunks):
                fillers.extend(proj_chunk_mms(g_, ti, c))
                if i > 0:
                    fillers.append(lambda: rope_tail(False))
            fillers.append(lambda: rope_tail(False))

            def pump(n):
                for _ in range(n):
                    if fillers:
                        fillers.pop(0)()

            # ---------------- attention ----------------
            def attention_block(h, ch, fill_rate, tail=None, pre=None, add_fillers=None):
                """One (head, chunk): 16 sk tiles of scores->exp->attnv.
                Returns the block's normalization as a closure; the caller
                passes it as `pre` to the NEXT block, which emits it after
                its second exp — so the exp stream never pauses at block
                transitions."""
                g, lo = h // 2, (h % 2 == 0)
                qz = qzlo[g] if lo else qzhi[g]
                oT = ops.tile([65, SQ], f32, tag="oT", name="oT")
                exps = []
                n_sk = S // 128

                def attnv(sk):
                    ej = exps[sk]
                    for n in range(SQ // 512):
                        nsl = slice(512 * n, 512 * (n + 1))
                        nc.tensor.matmul(
                            oT[:, nsl],
                            v_c[:, sk, h, :],
                            ej[:, nsl],
                            start=(sk == 0),
                            stop=(sk == n_sk - 1),
                        )

                for sk in range(n_sk):
                    s = sps.tile([128, SQ], f32, tag="s", name="s")
                    sksl = slice(128 * sk, 128 * (sk + 1))
                    for n in range(SQ // 512):
                        gsl = slice(SQ * ch + 512 * n, SQ * ch + 512 * (n + 1))
                        nc.tensor.matmul(
                            s[:, 512 * n : 512 * (n + 1)],
                            kT[g][:, sksl],
                            qz[:, gsl],
                            start=True,
                            stop=True,
                        )
                    e_t = expp.tile([128, SQ], bf16, tag="e")
                    nc.scalar.activation(e_t[:], s[:], AF.Exp, scale=0.125)
                    exps.append(e_t)
                    if sk == 1:
                        if pre is not None:
                            pre()
                        if add_fillers:
                            fillers.extend(add_fillers)
                    # lag-2: keeps the scores->exp->attnv chain off the
                    # critical path (PE stays back-to-back)
                    if sk > 1:
                        attnv(sk - 2)
                    pump(fill_rate)
                attnv(n_sk - 2)
                attnv(n_sk - 1)

                # normalize: evac oT, broadcast denom row via K=1 matmul,
                # reciprocal, multiply
                if tail is None:
                    def norm_closure():
                        oX = nmp.tile([65, SQ], bf16, tag="oX", name="oX")
                        nc.vector.tensor_copy(oX[:], oT[:])
                        aoB = None
                        if not lo:
                            aoB = nmp.tile([64, SQ], bf16, tag="aoB", name="aoB")
                        cslice = slice(SQ * ch, SQ * (ch + 1))
                        for n in range(SQ // 512):
                            nsl = slice(512 * n, 512 * (n + 1))
                            csl = slice(SQ * ch + 512 * n, SQ * ch + 512 * (n + 1))
                            db = mps.tile([64, 512], f32, tag="m", name="db")
                            nc.tensor.matmul(
                                db[:], onesrow[64:65, :], oX[64:65, nsl],
                                start=True, stop=True,
                            )
                            rbn = nmp.tile([64, 512], f32, tag="rb", name="rbn")
                            nc.vector.reciprocal_approx_fast(rbn[:], db[:])
                            if lo:
                                nc.vector.tensor_mul(
                                    att_o[g][0:64, csl], oX[0:64, nsl], rbn[:]
                                )
                            else:
                                nc.vector.tensor_mul(aoB[:, nsl], oX[0:64, nsl], rbn[:])
                        if not lo:
                            nc.sync.dma_start(att_o[g][64:128, cslice], aoB[:])

                    return norm_closure
                else:
                    # last block: lowest-latency norm per 512-half (PE
                    # broadcast matmul), with dependent work interleaved
                    for n in range(SQ // 512):
                        nsl = slice(512 * n, 512 * (n + 1))
                        csl = slice(SQ * ch + 512 * n, SQ * ch + 512 * (n + 1))
                        oXh = nmp.tile([65, 512], bf16, tag="oXh")
                        nc.scalar.copy(oXh[:], oT[:, nsl])
                        db = mps.tile([64, 512], f32, tag="m", name="db")
                        nc.tensor.matmul(
                            db[:], onesrow[64:65, :], oXh[64:65, :],
                            start=True, stop=True,
                        )
                        rbh = nmp.tile([64, 512], f32, tag="rbh")
                        nc.vector.reciprocal_approx_fast(rbh[:], db[:])
                        if lo:
                            # lo head: write att_o directly, no DMA hop
                            nc.vector.tensor_mul(
                                att_o[g][0:64, csl], oXh[0:64, :], rbh[:]
                            )
                        else:
                            aoh = nmp.tile([64, 512], bf16, tag="aoh")
                            nc.vector.tensor_mul(aoh[:], oXh[0:64, :], rbh[:])
                            # scalar queue: the sync queue carries the out
                            # DMAs that the interleaved outproj slabs emit
                            nc.scalar.dma_start(att_o[g][64:128, csl], aoh[:])
                        for cb in tail[n]:
                            cb()

            def outproj_chunk(st):
                """Closures for one 128-row slab of the output projection."""
                ssl = slice(128 * st, 128 * (st + 1))
                obox = {}
                ops_ = []
                for half in range(2):
                    nsl = slice(512 * half, 512 * (half + 1))
                    box = {}

                    def mm(g, box=box, nsl=nsl, half=half):
                        def run():
                            if g == 0:
                                box["op"] = mps.tile(
                                    [128, 512], f32, tag="m", name=f"op{st}_{half}"
                                )
                            nc.tensor.matmul(
                                box["op"][:],
                                att_o[g][:, ssl],
                                wo_sb[g][:, nsl],
                                start=(g == 0),
                                stop=(g == PAIRS - 1),
                            )

                        return run

                    def evac(box=box, nsl=nsl, half=half, obox=obox):
                        if half == 0:
                            obox["ot"] = osbp.tile([128, E], bf16, tag="ot", name="ot")
                        ot = obox["ot"]
                        # ACT only once exp is done (st >= 8 runs in the tail)
                        if st >= 8 and (st + half) % 2 == 0:
                            nc.scalar.copy(ot[:, nsl], box["op"][:])
                        else:
                            nc.vector.tensor_copy(ot[:, nsl], box["op"][:])
                        if half == 1:
                            nc.sync.dma_start(out[ssl, :], ot[:])

                    ops_.extend([mm(g) for g in range(PAIRS)] + [evac])
                return ops_

            # ch-major block order: all heads at sq 0:1024 first, so the
            # first output-projection half (and its DMA) overlaps the second
            # half of attention.  Pair-1 projection MMs are pumped into the
            # exp-paced gaps of the first two blocks.
            # pair-0's four blocks run first: their 64 exp-paced
            # iterations absorb pair-1's projection evenly (the old order
            # forced it into 32 iterations, going PE-bound)
            pn = attention_block(0, 0, 2)
            pn = attention_block(1, 0, 1, pre=pn)
            pn = attention_block(0, 1, 1, pre=pn)
            pn = attention_block(1, 1, 1, pre=pn)
            while fillers:
                pump(1)
            pn = attention_block(2, 0, 0, pre=pn)
            pn = attention_block(3, 0, 0, pre=pn)
            # outproj for sq rows 0:1024: extended inside (3,1) right after
            # (3,0)'s deferred norm is emitted at its sk==1
            och0 = [op_ for st in range(8) for op_ in outproj_chunk(st)]
            pn = attention_block(3, 1, 2, pre=pn, add_fillers=och0)
            # last block is (2,1), a lo head: its norm writes att_o directly
            # (no DMA hop).  outproj slabs for rows 1024:2048 interleave
            # behind each 512-wide normalized half
            tail = [
                [op_ for st in range(8, 12) for op_ in outproj_chunk(st)],
                [op_ for st in range(12, 16) for op_ in outproj_chunk(st)],
            ]
            attention_block(2, 1, 2, tail=tail, pre=pn)
            while fillers:
                pump(1)
            _psum_stack.close()

    nc.compile()
    return nc


def _get_program():
    if "nc" not in _BUILT:
        _BUILT["nc"] = _build_program()
    return _BUILT["nc"]


def _host_inputs(x, W_qkv, W_out):
    """Build the 8 per-core input maps."""
    import ml_dtypes

    bf = ml_dtypes.bfloat16
    f = np.float32
    x = np.asarray(x, dtype=f)
    W_qkv = np.asarray(W_qkv, dtype=f)
    W_out = np.asarray(W_out, dtype=f)

    inv_freq = 1.0 / (ROPE_THETA ** (np.arange(0, D, 2, dtype=np.float64) / D))
    p = np.arange(128)
    freq_row = inv_freq[(p % D) // 2]  # [128]
    ang = freq_row[:, None] * np.arange(S, dtype=np.float64)[None, :]  # [128, S]
    cos_t = np.cos(ang).astype(bf)
    sign = np.where(p % 2 == 0, -1.0, 1.0)[:, None]
    sin_t = (np.sin(ang) * sign).astype(bf)

    msw = np.zeros((128, 128), dtype=bf)
    msw[p, p ^ 1] = 1.0

    maps = []
    for core in range(N_CORES):
        b, hg = divmod(core, HG)
        hs = [HPG * hg + i for i in range(HPG)]
        w_qk = np.concatenate(
            [W_qkv[:, h * D : (h + 1) * D] for h in hs]
            + [W_qkv[:, ATT + h * D : ATT + (h + 1) * D] for h in hs],
            axis=1,
        )
        w_v = np.concatenate(
            [W_qkv[:, 2 * ATT + h * D : 2 * ATT + (h + 1) * D] for h in hs], axis=1
        )
        w_v = np.ascontiguousarray(
            w_v.reshape(E // 128, 128, HPG * D).transpose(1, 0, 2)
        )
        w_o = np.concatenate([W_out[h * D : (h + 1) * D, :] for h in hs], axis=0)
        maps.append(
            {
                "xT": np.ascontiguousarray(x[b].T).astype(bf),
                "w_qk": np.ascontiguousarray(w_qk).astype(bf),
                "w_v": np.ascontiguousarray(w_v).astype(bf),
                "w_o": np.ascontiguousarray(w_o).astype(bf),
                "cos_t": cos_t,
                "sin_t": sin_t,
                "mswap": msw,
                "ones_in": np.ones((1, 64), dtype=bf),
            }
        )
    return maps


def kernel(x, W_qkv, W_out):
    from concourse.bass_utils import run_bass_kernel_spmd

    nc = _get_program()
    maps = _host_inputs(x, W_qkv, W_out)
    res = run_bass_kernel_spmd(nc, maps, core_ids=list(range(N_CORES)))
    out = np.zeros((B, S, E), dtype=np.float32)
    for core in range(N_CORES):
        b = core // HG
        out[b] += res.results[core]["out"].astype(np.float32)
    return out
